# revision 36
# baseline (speedup 1.0000x reference)
# Depthwise causal conv1d (B=8, T=4096, C=1024, K=4, dilation=1) on 8 TRN2
# NeuronCores.
#
# Math: y[b, t, c] = sum_{j=0..3} weight[c, 3-j] * x[b, t-j, c]   (x[t<0] = 0)
#
# Strategy (v11 — fp16 I/O, PE+DVE split, weights ride inside x):
#   - Shard batch: core b handles x[b] (one full (T, C) slice).
#   - Host transposes each shard to (C, T), casts to fp16, and prefixes
#     every row with [4 taps | 128 identity cols | 4 zero halo cols], so a
#     block's first x load delivers its per-partition tap scalars AND the
#     identity needed to build diagonal matmul weights — no separate const
#     DMAs serializing at the ring head, and nothing for compute to wait
#     on except its own x tile.
#   - fp16 halves DMA traffic vs fp32 (~8.5MB in + 8.4MB out per core,
#     ~360 GB/s per-core roofline shared by loads+stores).  fp16 keeps 11
#     significand bits: worst-case abs err ~1e-2 vs output scale ~3.2
#     (gate is 2e-2 relative).
#   - Per 128-channel block, 8 512-col subtiles split 5 PE / 3 DVE:
#       * PE: 4 accumulating matmuls per subtile against fp16 diagonal
#         weights (PSUM sums the taps); ACT copies PSUM->SBUF two banks
#         per op with an inline fp32->fp16 cast.
#       * DVE: one slab of 4 tensor_scalar mults (4x_2p) + 3
#         tensor_tensor adds (2x_1p).
#   - Diagonal weights: DVE builds blocks 0-1 up front, ACT builds the
#     rest two blocks ahead (activation-copy of the in-x identity scaled
#     per partition).  GpSimd stays idle: both its tensor_scalar and its
#     dependency latency wreck the pipeline (measured).
#   - x loads ride the SP HWDGE ring as ~0.5MB pieces (block 0: quarters);
#     y stores ride the ACT ring as halves, the final one split smaller to
#     shorten the drain tail.  bufs=8 queues all loads up front.
#   - A few discarded matmuls ramp the PE p-state (cold PE runs at ~half
#     clock; promotion needs ~10us of sustained activity).
#   - Blocks 0 and 7 run DVE-first/PE-last: block 0 so DVE starts off the
#     first x quarter, block 7 so the final store hangs off the short
#     PE->ACT chain.

import numpy as np

B, T, C, K = 8, 4096, 1024, 4
N_CORES = 8
P = 128  # SBUF partitions
NSUB = 512  # PE subtile width (one fp32 PSUM bank)
HALO = 4  # zero columns between the const prefix and x (causal left pad)
D0 = P + 2 * K + HALO  # per-row prefix: ident, taps (K fp32 as 2K fp16), halo
PE_SUB = 5  # PE subtiles per block (of 8); the other 3 go to DVE
N_WARMUP = 6  # discarded matmuls to ramp the PE p-state during DMA latency

_CACHE = {}


def _build_nc():
    import concourse.mybir as mybir
    import concourse.tile as tile
    from concourse import bacc

    f32 = mybir.dt.float32
    f16 = mybir.dt.float16
    add = mybir.AluOpType.add
    ncb = C // P  # channel blocks per core
    half = T // 2

    nc = bacc.Bacc(None)
    # x rows: [ident(128) | w0..w3 as fp32 bytes (8 fp16 cols) | 0*4 | x(4096)]
    # blocks 1..7 skip the ident columns on load; builds reuse block 0's.
    x = nc.declare_dram_parameter("x", [C, T + D0], f16, isOutput=False)
    y = nc.declare_dram_parameter("y", [C, T], f16, isOutput=True)

    # dram col of x[t] is D0 + t; piece boundaries in dram cols:
    q = half // 2
    A0_HI = D0 + q  # xa0: [0, A0_HI)
    A1_LO = A0_HI - HALO  # xa1: [A1_LO, TA_HI)
    TA_HI = D0 + half  # xta: [0, TA_HI)
    TB_LO = TA_HI - HALO  # xtb: [TB_LO, D0 + T)

    with tile.TileContext(nc) as tc:
        with (
            tc.tile_pool(name="const", bufs=1) as cpool,
            tc.tile_pool(name="xin", bufs=8) as xpool,
            tc.tile_pool(name="yout", bufs=8) as ypool,
            tc.tile_pool(name="tmp", bufs=2) as tpool,
            tc.tile_pool(name="ps", bufs=3, space="PSUM") as pspool,
        ):
            # PE p-state warm-up (results discarded).
            scratch = cpool.tile([P, NSUB], f16)
            nc.gpsimd.memset(scratch[:, :], 0.0)
            for _ in range(N_WARMUP):
                psw = pspool.tile([P, NSUB], f32, tag="warm", bufs=1, name="psw")
                nc.tensor.matmul(
                    psw[:, :], scratch[:, :P], scratch[:, :], start=True, stop=True
                )

            wdiag = {}  # (cb, j) -> [P, P] diag(weight[cb*128+p, K-1-j])
            first_part = [None] * ncb  # x piece holding each block's prefix
            tap_off = [None] * ncb  # tile-local col of the taps
            ident_ap = [None]  # block 0's identity columns (persistent tile)

            def wcol_of(cb, j):
                # fp32 tap scalar: 2 fp16 cols bitcast back to one fp32
                c2 = tap_off[cb] + 2 * (K - 1 - j)
                return first_part[cb][:, c2 : c2 + 2].bitcast(f32)

            def build_wdiag(cb, eng):
                for j in range(K):
                    wd = cpool.tile([P, P], f16, tag=f"wd_{cb}_{j}", name="wd")
                    if eng == "dve":
                        nc.vector.tensor_scalar_mul(
                            out=wd[:, :],
                            in0=ident_ap[0],
                            scalar1=wcol_of(cb, j),
                        )
                    else:
                        nc.scalar.mul(wd[:, :], ident_ap[0], wcol_of(cb, j))
                    wdiag[(cb, j)] = wd

            x_parts = [None] * ncb

            def load_x(cb):
                rows = slice(cb * P, (cb + 1) * P)
                if cb == 0:
                    xa0 = xpool.tile([P, A0_HI], f16, tag="xa0", bufs=1)
                    xa1 = xpool.tile([P, TA_HI - A1_LO], f16, tag="xa1", bufs=1)
                    nc.sync.dma_start(out=xa0[:, :], in_=x[rows, :A0_HI])
                    nc.sync.dma_start(out=xa1[:, :], in_=x[rows, A1_LO:TA_HI])
                    parts = [(0, A0_HI, xa0), (A1_LO, TA_HI, xa1)]
                else:
                    xta = xpool.tile([P, TA_HI - P], f16, tag="xta")
                    nc.sync.dma_start(out=xta[:, :], in_=x[rows, P:TA_HI])
                    parts = [(P, TA_HI, xta)]
                xtb = xpool.tile([P, D0 + T - TB_LO], f16, tag="xtb")
                nc.sync.dma_start(out=xtb[:, :], in_=x[rows, TB_LO : D0 + T])
                parts.append((TB_LO, D0 + T, xtb))
                x_parts[cb] = parts
                first_part[cb] = parts[0][2]
                tap_off[cb] = P if cb == 0 else 0
                if cb == 0:
                    ident_ap[0] = parts[0][2][:, :P]

            load_x(0)
            build_wdiag(0, "dve")
            if ncb > 1:
                load_x(1)
                build_wdiag(1, "dve")

            for cb in range(ncb):
                flip = cb == 0 or cb == ncb - 1
                last = cb == ncb - 1
                rows = slice(cb * P, (cb + 1) * P)
                if cb + 2 < ncb:
                    load_x(cb + 2)
                    build_wdiag(cb + 2, "act")
                parts = x_parts[cb]
                xt0 = first_part[cb]

                def x_ap(lo, hi):  # dram cols [lo, hi)
                    for plo, phi, t in parts:
                        if lo >= plo and hi <= phi:
                            return t[:, lo - plo : hi - plo]
                    raise AssertionError((lo, hi))

                yt0 = ypool.tile([P, half], f16, tag="yt0")
                yt1 = ypool.tile([P, half], f16, tag="yt1")

                def y_ap(lo, hi):  # y cols [lo, hi)
                    if hi <= half:
                        return yt0[:, lo:hi]
                    assert lo >= half
                    return yt1[:, lo - half : hi - half]

                # Layout: normal blocks PE 0..4 / DVE 5..7, flipped blocks
                # DVE 0..2 / PE 3..7.  Slab first so stores in the PE
                # section pick up its tile deps.  Stores: 'h0' = y[:half],
                # 'h1' = y[half:], 'h1a'/'h1b' split the last block's tail.
                if flip:
                    slabs = (
                        [(0, 1024), (1024, 512)] if cb == 0 else [(0, 1536)]
                    )
                    if last:
                        pe_groups = [
                            ((3,), "h0"),
                            ((4, 5), "h1a"),
                            ((6,), "h1c"),
                            ((7,), "h1b"),
                        ]
                    else:
                        pe_groups = [((3,), "h0"), ((4, 5), None), ((6, 7), "h1")]
                else:
                    slabs = [(PE_SUB * NSUB, (8 - PE_SUB) * NSUB)]
                    pe_groups = [((0, 1), None), ((2, 3), "h0"), ((4,), "h1")]

                # --- DVE slab: y[:, s:s+L] = sum_j w_j * x[:, s-j:s-j+L] ---
                for s, L in slabs:

                    def xoff(j):
                        off = D0 + s - j
                        return x_ap(off, off + L)

                    wc = [wcol_of(cb, j) for j in range(K)]
                    a = tpool.tile([P, L], f16, tag="a")
                    bb = tpool.tile([P, L], f16, tag="b")
                    cc = tpool.tile([P, L], f16, tag="c")
                    dd = tpool.tile([P, L], f16, tag="d")
                    nc.vector.tensor_scalar_mul(out=a[:, :], in0=xoff(0), scalar1=wc[0])
                    nc.vector.tensor_scalar_mul(
                        out=bb[:, :], in0=xoff(1), scalar1=wc[1]
                    )
                    nc.vector.tensor_tensor(
                        out=a[:, :], in0=a[:, :], in1=bb[:, :], op=add
                    )
                    nc.vector.tensor_scalar_mul(
                        out=cc[:, :], in0=xoff(2), scalar1=wc[2]
                    )
                    nc.vector.tensor_scalar_mul(
                        out=dd[:, :], in0=xoff(3), scalar1=wc[3]
                    )
                    nc.vector.tensor_tensor(
                        out=cc[:, :], in0=cc[:, :], in1=dd[:, :], op=add
                    )
                    nc.vector.tensor_tensor(
                        out=y_ap(s, s + L), in0=a[:, :], in1=cc[:, :], op=add
                    )
                # --- PE groups (1-2 subtiles per PSUM tile, one ACT copy) ---
                for ms, store in pe_groups:
                    n = len(ms)
                    tag = "pp" if n == 2 else "sg"
                    ps = pspool.tile(
                        [P, n * NSUB], f32, tag=tag, bufs=2 if n == 2 else 2
                    )
                    for i, m in enumerate(ms):
                        for j in range(K):
                            off = D0 + NSUB * m - j
                            nc.tensor.matmul(
                                ps[:, i * NSUB : (i + 1) * NSUB],
                                wdiag[(cb, j)][:, :],
                                x_ap(off, off + NSUB),
                                start=(j == 0),
                                stop=(j == K - 1),
                            )
                    lo = NSUB * ms[0]
                    nc.scalar.copy(y_ap(lo, lo + n * NSUB), ps[:, :])
                    if store == "h0":
                        nc.scalar.dma_start(out=y[rows, :half], in_=yt0[:, :])
                    elif store == "h1":
                        nc.scalar.dma_start(out=y[rows, half:], in_=yt1[:, :])
                    elif store == "h1a":
                        nc.scalar.dma_start(
                            out=y[rows, half : half + 1024], in_=yt1[:, :1024]
                        )
                    elif store == "h1c":
                        nc.scalar.dma_start(
                            out=y[rows, half + 1024 : half + 1536],
                            in_=yt1[:, 1024:1536],
                        )
                    elif store == "h1b":
                        nc.scalar.dma_start(
                            out=y[rows, half + 1536 :], in_=yt1[:, 1536:]
                        )
    return nc


def _get_nc():
    if "nc" not in _CACHE:
        nc = _build_nc()
        nc.finalize()
        _CACHE["nc"] = nc
    return _CACHE["nc"]


def _prep_inputs(x, weight):
    x = np.asarray(x)
    w32 = np.ascontiguousarray(np.asarray(weight, dtype=np.float32))  # (C, K)
    ncb = C // P
    eye = np.eye(P, dtype=np.float16)
    ident_rows = np.tile(eye, (ncb, 1))  # (C, P): row c -> eye[c % P]
    in_maps = []
    for b in range(N_CORES):
        xt = np.zeros((C, T + D0), dtype=np.float16)
        xt[:, :P] = ident_rows
        xt[:, P : P + 2 * K] = w32.view(np.float16)
        xt[:, D0:] = x[b].T
        in_maps.append({"x": xt})
    return in_maps


def _collect_output(res):
    y = np.empty((B, T, C), dtype=np.float32)
    for b in range(N_CORES):
        y[b] = res.results[b]["y"].T.astype(np.float32)
    return y


LAST_RESULT = None


def kernel(x, weight):
    global LAST_RESULT
    from concourse.bass_utils import run_bass_kernel_spmd

    in_maps = _prep_inputs(x, weight)
    nc = _get_nc()
    res = run_bass_kernel_spmd(nc, in_maps, list(range(N_CORES)))
    LAST_RESULT = res
    return _collect_output(res)
